# revision 36
# baseline (speedup 1.0000x reference)
"""BiLSTM-CRF loss kernel for Trainium2, data-parallel over batch on 8 NeuronCores.

Per-core program (B_local=16 sequences, S=512, T=20 tags, E=100, H=128):
  Host prep: embeddings pre-gathered/transposed to xsT [E+1, S*16] bf16 (ones
  row folds the input-projection bias into the matmul); weight layouts
  transposed, gate order (i,f,o,g); the g block and whh/wout carry extra
  factors of 2 so the cell uses only Sigmoid activations and stores h/2.

  Main loop: 512-step fwd+bwd LSTM recurrence (two independent dependency
  chains) with producer work streamed in as background items between steps:
  xsT piece DMAs (on the Pool engine's DMA queue, off the Sync queue), the
  one-hot of tags, and (second half, middle-outward as hf/hb become
  available) the emission chunks em = W_out @ [hf;hb], expE = exp(em +
  b_out), and the CRF numerator pieces.

  Per step per dir: 4 small input-projection matmuls (K=E+1, N=16) write the
  next step's gates into a fresh PSUM bank one step ahead (they execute
  inside the h-wait stall on the PE); 4 W_hh matmuls then accumulate on top;
  one Sigmoid covers all gates (tanh(g) = 2*sig(2g)-1); cell update
  k = si*(s2g-1/2), c = 2k + sf*c_prev; tanh(c) via sig(2c);
  h' = h/2 = so*(sig(2c)-1/2). Measured cycle ~1.83us/step, at the
  dependency-chain floor for this cell structure.

  Tail: emission chunks 0/15, then the CRF partition function as two serial
  chains meeting in the middle: alpha (t=0..255) and gamma_t = E_t * beta_t
  (t=511..256), renormalized by the compile-time constant 2^-69 every 16
  steps (exact power of two, no data-dependent renorm work); the log2
  bookkeeping is added back as a constant in the final combine.

mask is all ones for this problem (spec fill=ones), so masking is elided and
seq_ends = S-1.
"""

import math
import os
import sys

import numpy as np

sys.path.insert(0, "/opt/trn_rl_repo")

import concourse.bass as bass  # noqa: F401 (registers bass types used by tile)
import concourse.mybir as mybir
import concourse.tile as tile
from concourse import bacc
from concourse.masks import make_identity

AF = mybir.ActivationFunctionType
ALU = mybir.AluOpType
AX = mybir.AxisListType
F32 = mybir.dt.float32
BF16 = mybir.dt.bfloat16
I32 = mybir.dt.int32

V, T, E, HD = 32000, 20, 100, 256
H = 128
B, S = 128, 512
NCORES = 8
BL = B // NCORES          # 16 sequences per core
TB = S * BL               # 8192 tokens per core
CHS = 32                  # time steps per projection/emission chunk
NPC = S // CHS            # 16 chunks
RENORM = 16               # DP renorm period (steps)
RSH = 69                  # A *= 2^-69 each renorm (~20^16)
DPH = S // 2              # alpha/gamma half length


def build_program():
    nc = bacc.Bacc(None, target_bir_lowering=False)

    # ---- DRAM I/O ----
    tags_d = nc.dram_tensor("tags_tb", [1, TB], BF16, kind="ExternalInput")
    CW = CHS * BL
    xsT_d = nc.dram_tensor("xsT_in", [E + 1, TB], BF16, kind="ExternalInput")
    wih_f_d = nc.dram_tensor("wihT_f", [E + 1, 4 * H], BF16, kind="ExternalInput")
    wih_b_d = nc.dram_tensor("wihT_b", [E + 1, 4 * H], BF16, kind="ExternalInput")
    whh_f_d = nc.dram_tensor("whhT_f", [H, 4 * H], BF16, kind="ExternalInput")
    whh_b_d = nc.dram_tensor("whhT_b", [H, 4 * H], BF16, kind="ExternalInput")
    wout_d = nc.dram_tensor("woutT", [H, 2 * T], BF16, kind="ExternalInput")
    bout_d = nc.dram_tensor("b_out_c", [T, 1], F32, kind="ExternalInput")
    start_d = nc.dram_tensor("start_c", [T, 1], F32, kind="ExternalInput")
    end_d = nc.dram_tensor("end_c", [T, 1], F32, kind="ExternalInput")
    trans_d = nc.dram_tensor("trans", [T, T], F32, kind="ExternalInput")
    transT_d = nc.dram_tensor("transT", [T, T], F32, kind="ExternalInput")
    trans_bf_d = nc.dram_tensor("trans_bf", [T, T], BF16, kind="ExternalInput")
    out_d = nc.dram_tensor("out", [1, BL], F32, kind="ExternalOutput")
    DBG = bool(int(os.environ.get("BASS_KERNEL_DEBUG", "0")))
    if DBG:
        dbg_hf = nc.dram_tensor("dbg_hf", [128, 4 * BL], F32, kind="ExternalOutput")
        dbg_hb = nc.dram_tensor("dbg_hb", [128, 4 * BL], F32, kind="ExternalOutput")
        dbg_ee = nc.dram_tensor("dbg_ee", [T, 4 * BL], F32, kind="ExternalOutput")
        dbg_na = nc.dram_tensor("dbg_na", [T, BL], F32, kind="ExternalOutput")
        dbg_al = nc.dram_tensor("dbg_al", [T, BL], F32, kind="ExternalOutput")
        dbg_ga = nc.dram_tensor("dbg_ga", [T, BL], F32, kind="ExternalOutput")
        dbg_oh = nc.dram_tensor("dbg_oh", [T, 4 * BL], F32, kind="ExternalOutput")
        dbg_fin = nc.dram_tensor("dbg_fin", [T, BL], F32, kind="ExternalOutput")
        dbg_lnf = nc.dram_tensor("dbg_lnf", [1, BL], F32, kind="ExternalOutput")
        dbg_scp = nc.dram_tensor("dbg_scp", [1, BL], F32, kind="ExternalOutput")

    with tile.TileContext(nc) as tc:
        with tc.tile_pool(name="persist", bufs=1) as pp:
            # ---- persistent SBUF tiles ----
            xsT = pp.tile([E + 1, TB], BF16, tag="xsT")
            hf = pp.tile([128, TB], BF16, tag="hf")
            hb = pp.tile([128, TB], BF16, tag="hb")
            expE = pp.tile([T, TB], F32, tag="expE")
            esc = pp.tile([T, 2 * CHS * BL], F32, tag="esc")  # 2^-69-scaled slices
            oh = pp.tile([T, TB], BF16, tag="oh")
            wih_f = pp.tile([E + 1, 4 * H], BF16, tag="wihf")
            wih_b = pp.tile([E + 1, 4 * H], BF16, tag="wihb")
            whh_f = pp.tile([H, 4 * H], BF16, tag="whhf")
            whh_b = pp.tile([H, 4 * H], BF16, tag="whhb")
            wout = pp.tile([H, 2 * T], BF16, tag="wout")
            bout = pp.tile([T, 1], F32, tag="bout")
            start_t = pp.tile([T, 1], F32, tag="start")
            end_t = pp.tile([T, 1], F32, tag="end")
            trans_bf = pp.tile([T, T], BF16, tag="transbf")
            expT = pp.tile([T, T], F32, tag="expT")
            expTT = pp.tile([T, T], F32, tag="expTT")
            exp_end = pp.tile([T, 1], F32, tag="expend")
            exp_start = pp.tile([T, 1], F32, tag="expstart")
            identb = pp.tile([128, 128], BF16, tag="identb")
            ones_t1 = pp.tile([T, 1], F32, tag="onest1")
            rs_t1 = pp.tile([T, 1], F32, tag="rst1")  # 2^-69 column for the
            # final colsum so Ln's input lands in a sane range
            ones_1t = pp.tile([1, T], BF16, tag="ones1t")
            iot_f = pp.tile([T, 1], F32, tag="iotf")
            num_acc = pp.tile([T, BL], F32, tag="numacc")

            # ---- param loads & constants (spread across engine DMA queues
            # so the first recurrence steps aren't gated on one serial queue) ----
            tmp_tr = pp.tile([T, T], F32, tag="tmptr")
            for eng, sb, d in [
                (nc.sync, wih_f, wih_f_d), (nc.scalar, wih_b, wih_b_d),
                (nc.gpsimd, whh_f, whh_f_d), (nc.scalar, whh_b, whh_b_d),
                (nc.sync, wout, wout_d), (nc.sync, bout, bout_d),
                (nc.sync, start_t, start_d), (nc.sync, end_t, end_d),
                (nc.gpsimd, trans_bf, trans_bf_d),
            ]:
                eng.dma_start(out=sb[:], in_=d[:])
            nc.scalar.dma_start(out=tmp_tr[:], in_=trans_d[:])
            nc.scalar.activation(expT[:], tmp_tr[:], AF.Exp)
            tmp_tr2 = pp.tile([T, T], F32, tag="tmptr2")
            nc.scalar.dma_start(out=tmp_tr2[:], in_=transT_d[:])
            nc.scalar.activation(expTT[:], tmp_tr2[:], AF.Exp)
            nc.scalar.activation(exp_end[:], end_t[:], AF.Exp)
            nc.scalar.activation(exp_start[:], start_t[:], AF.Exp)
            make_identity(nc, identb[:])
            nc.vector.memset(ones_t1[:], 1.0)
            nc.vector.memset(rs_t1[:], float(2.0 ** (-RSH)))
            nc.vector.memset(ones_1t[:], 1.0)
            iot_i = pp.tile([T, 1], I32, tag="ioti")
            nc.gpsimd.iota(iot_i[:], pattern=[[0, 1]], base=0, channel_multiplier=1)
            nc.vector.tensor_copy(iot_f[:], iot_i[:])
            nc.vector.memset(num_acc[:], 0.0)

            with (
                tc.tile_pool(name="gat_sb", bufs=3) as gsb,
                tc.tile_pool(name="wide_ps", bufs=1, space="PSUM") as wps,
                tc.tile_pool(name="g_ps", bufs=3, space="PSUM") as gps_pool,
                tc.tile_pool(name="p_sb", bufs=3) as psb,
                tc.tile_pool(name="cell_sb", bufs=4) as csb,
                tc.tile_pool(name="em_sb", bufs=3) as esb,
                tc.tile_pool(name="dp_ps", bufs=2, space="PSUM") as dps,
                tc.tile_pool(name="dp_sb", bufs=3) as dsb,
            ):
                # ---------- background item emitters ----------

                def emit_xchunk(c):
                    # one sixteenth of xsT (exactly proj chunk c's columns);
                    # issued from the idle Pool engine's DMA queue so these
                    # transfers don't serialize behind the Sync queue
                    cs = slice(c * TB // 16, (c + 1) * TB // 16)
                    nc.gpsimd.dma_start(out=xsT[:, cs], in_=xsT_d[:, cs])

                def emit_oh(c):
                    # one-hot of tags for chunk c (tags only; no recurrence dep)
                    cs = slice(c * CHS * BL, (c + 1) * CHS * BL)
                    tgc = gsb.tile([1, CW], BF16, tag="tgc", name="tgc")
                    nc.gpsimd.dma_start(out=tgc[:], in_=tags_d[:, cs])
                    wtile = wps.tile([128, 1024], BF16, tag="wide", name="wtile")
                    ohp = wtile[0:T, :].bitcast(F32)
                    nc.tensor.matmul(
                        ohp, lhsT=ones_1t[:], rhs=tgc[:],
                        start=True, stop=True,
                    )
                    nc.vector.tensor_tensor(
                        out=oh[:, cs], in0=ohp,
                        in1=iot_f[:].to_broadcast([T, CHS * BL]), op=ALU.is_equal,
                    )

                RS = float(2.0 ** (-RSH))
                # ---------- emission-chunk items ----------
                na_ng = [0, 0]

                def em_items(c):
                    CWc = CHS * BL
                    cs = slice(c * CWc, (c + 1) * CWc)
                    st = {}

                    def i_mm():
                        wtile = wps.tile([128, 1024], BF16, tag="wide",
                                         name="wtile")
                        emp = wtile[0:T, :].bitcast(F32)
                        st["emp"] = emp
                        h2 = CWc // 2
                        cs_a = slice(c * CWc, c * CWc + h2)
                        nc.tensor.matmul(
                            emp[:, :h2], lhsT=wout[:, 0:T], rhs=hf[:, cs_a],
                            start=True, stop=False,
                        )
                        nc.tensor.matmul(
                            emp[:, :h2], lhsT=wout[:, T : 2 * T], rhs=hb[:, cs_a],
                            start=False, stop=True,
                        )

                    def i_mm_b():
                        h2 = CWc // 2
                        cs_b = slice(c * CWc + h2, (c + 1) * CWc)
                        emp = st["emp"]
                        nc.tensor.matmul(
                            emp[:, h2:], lhsT=wout[:, 0:T], rhs=hf[:, cs_b],
                            start=True, stop=False,
                        )
                        nc.tensor.matmul(
                            emp[:, h2:], lhsT=wout[:, T : 2 * T], rhs=hb[:, cs_b],
                            start=False, stop=True,
                        )

                    def i_exp():
                        # expE = exp(em + b_out) (bias folded into activation)
                        nc.scalar.activation(expE[:, cs], st["emp"], AF.Exp,
                                             bias=bout[:])

                    def i_esc():
                        for sI in range(c * CHS, (c + 1) * CHS):
                            if sI % RENORM == 0 and sI >= RENORM:
                                col = (sI // RENORM) * BL
                                nc.gpsimd.tensor_scalar_mul(
                                    esc[:, col : col + BL],
                                    expE[:, sI * BL : (sI + 1) * BL], RS,
                                )

                    def i_prod_a():
                        prod = esb.tile([T, CWc], F32, tag="prod", name="prod")
                        st["prod"] = prod
                        nc.vector.scalar_tensor_tensor(
                            out=prod[:, : CWc // 2], in0=st["emp"][:, : CWc // 2],
                            scalar=bout[:],
                            in1=oh[:, c * CWc : c * CWc + CWc // 2],
                            op0=ALU.add, op1=ALU.mult,
                        )

                    def i_prod_b():
                        nc.vector.scalar_tensor_tensor(
                            out=st["prod"][:, CWc // 2 :],
                            in0=st["emp"][:, CWc // 2 :], scalar=bout[:],
                            in1=oh[:, c * CWc + CWc // 2 : (c + 1) * CWc],
                            op0=ALU.add, op1=ALU.mult,
                        )

                    def i_red_a():
                        part = esb.tile([T, BL], F32, tag="part", name="part")
                        st["part"] = part
                        nc.vector.reduce_sum(
                            part[:],
                            st["prod"][:, : CWc // 2].rearrange(
                                "p (t b) -> p b t", b=BL
                            ),
                            axis=AX.X,
                        )

                    def i_red_b():
                        part_b = esb.tile([T, BL], F32, tag="part", name="part_b")
                        nc.vector.reduce_sum(
                            part_b[:],
                            st["prod"][:, CWc // 2 :].rearrange(
                                "p (t b) -> p b t", b=BL
                            ),
                            axis=AX.X,
                        )
                        nc.gpsimd.tensor_tensor(
                            out=num_acc[:], in0=num_acc[:], in1=st["part"][:],
                            op=ALU.add,
                        )
                        nc.gpsimd.tensor_tensor(
                            out=num_acc[:], in0=num_acc[:], in1=part_b[:],
                            op=ALU.add,
                        )

                    def i_trp():
                        wtile = wps.tile([128, 1024], BF16, tag="wide",
                                         name="wtile")
                        trp = wtile[0:T, :].bitcast(F32)
                        st["trp"] = trp
                        h2 = CWc // 2
                        nc.tensor.matmul(
                            trp[:, :h2], lhsT=trans_bf[:],
                            rhs=oh[:, c * CWc : c * CWc + h2],
                            start=True, stop=True,
                        )
                        nc.tensor.matmul(
                            trp[:, h2:], lhsT=trans_bf[:],
                            rhs=oh[:, c * CWc + h2 : (c + 1) * CWc],
                            start=True, stop=True,
                        )

                    def i_prod2_a():
                        npair = CHS if c < NPC - 1 else CHS - 1
                        st["npair"] = npair
                        prod2 = esb.tile([T, CWc], F32, tag="prod", name="prod2")
                        st["prod2"] = prod2
                        nc.vector.tensor_tensor(
                            out=prod2[:, : (npair * BL) // 2],
                            in0=st["trp"][:, : (npair * BL) // 2],
                            in1=oh[:, c * CWc + BL :][:, : (npair * BL) // 2],
                            op=ALU.mult,
                        )

                    def i_prod2_b():
                        npair = st["npair"]
                        h0 = (npair * BL) // 2
                        nc.vector.tensor_tensor(
                            out=st["prod2"][:, h0 : npair * BL],
                            in0=st["trp"][:, h0 : npair * BL],
                            in1=oh[:, c * CWc + BL + h0 :][
                                :, : npair * BL - h0
                            ],
                            op=ALU.mult,
                        )

                    def i_red2():
                        part2 = esb.tile([T, BL], F32, tag="part", name="part2")
                        nc.vector.reduce_sum(
                            part2[:],
                            st["prod2"][:, : st["npair"] * BL].rearrange(
                                "p (t b) -> p b t", b=BL
                            ),
                            axis=AX.X,
                        )
                        nc.gpsimd.tensor_tensor(
                            out=num_acc[:], in0=num_acc[:], in1=part2[:],
                            op=ALU.add,
                        )

                    return [i_mm, i_mm_b, i_exp, i_esc, i_prod_a, i_prod_b, i_red_a,
                            i_red_b, i_trp, i_prod2_a, i_prod2_b, i_red2]

                # ---------- background schedule ----------
                prologue = [("xch", 0), ("xch", 15)]
                windows = {i: [] for i in range(1, 16)}
                for i in range(1, 15):
                    # xsT piece i feeds proj chunk i (consumed from window i+1)
                    windows[i].append(("xch", i))
                    windows[i].append(("xch", 15 - i))
                for c in range(NPC):
                    windows[(c % 15) + 1].append(("oh", c))
                windows[16] = []
                # em chunks become computable middle-outward as hf/hb meet;
                # chunks 0 and 15 only at the very end (tail handles those)
                for wi, (ca, cb) in zip(
                    range(10, 17),
                    [(7, 8), (6, 9), (5, 10), (4, 11), (3, 12), (2, 13), (1, 14)],
                ):
                    for it in em_items(ca):
                        windows[wi].append(("emi", it))
                    for it in em_items(cb):
                        windows[wi].append(("emi", it))

                def run_item(item):
                    if item[0] == "xch":
                        emit_xchunk(item[1])
                    elif item[0] == "emi":
                        item[1]()
                    else:
                        emit_oh(item[1])

                wihs = [wih_f, wih_b]

                def emit_step_proj(dir_i, tt, final):
                    # input projection for step tt straight into a fresh gates
                    # PSUM bank; the whh matmuls later accumulate on top
                    gp = gps_pool.tile([128, 64], F32, tag="g", name="g_ps",
                                       space="PSUM")
                    for g in range(4):
                        nc.tensor.matmul(
                            gp[:, g * BL : (g + 1) * BL],
                            lhsT=wihs[dir_i][:, g * 128 : (g + 1) * 128],
                            rhs=xsT[:, tt * BL : (tt + 1) * BL],
                            start=True, stop=final,
                        )
                    return gp

                next_gps = [None, None]

                # PE p-state warm-up: ~5us of continuous dependency-free
                # matmuls on the on-device identity (no DMA input needed)
                # while the weight/xsT DMAs are still in flight, so the
                # Tensor engine reaches full clock before step 0
                warm = wps.tile([128, 1024], BF16, tag="wide", name="warm")
                wv = warm[:].bitcast(F32)[:, 0:128]
                for _ in range(24):
                    nc.tensor.matmul(wv, lhsT=identb[:], rhs=identb[:],
                                     start=True, stop=True)
                for item in prologue:
                    run_item(item)
                next_gps[0] = emit_step_proj(0, 0, True)
                next_gps[1] = emit_step_proj(1, S - 1, True)

                # ---------- main recurrence ----------
                c_slice = {0: None, 1: None}
                wq, wlen, qi = [], 0, 0
                for t in range(S):
                    if t % CHS == 0:
                        wq = windows.get(t // CHS + 1, [])
                        wlen, qi = len(wq), 0
                    # spread this window's items over its 32 steps
                    target = ((t % CHS) + 1) * wlen // CHS
                    while qi < target:
                        run_item(wq[qi])
                        qi += 1
                    tb_ = S - 1 - t
                    tts, whhs, hsts, hprevs, gpss, sigs = [], [], [], [], [], []
                    for dir_i in (0, 1):
                        if dir_i == 0:
                            tts.append(t)
                            whhs.append(whh_f)
                            hsts.append(hf)
                            hprevs.append(
                                None if t == 0 else hf[:, (t - 1) * BL : t * BL]
                            )
                        else:
                            tts.append(tb_)
                            whhs.append(whh_b)
                            hsts.append(hb)
                            hprevs.append(
                                None if t == 0
                                else hb[:, (tb_ + 1) * BL : (tb_ + 2) * BL]
                            )
                    for dir_i in (0, 1):
                        g_ps = next_gps[dir_i]
                        gpss.append(g_ps)
                        if t + 1 < S:
                            tt2 = t + 1 if dir_i == 0 else S - 2 - t
                            next_gps[dir_i] = emit_step_proj(dir_i, tt2, False)
                        if t > 0:
                            for g in range(4):
                                nc.tensor.matmul(
                                    g_ps[:, g * BL : (g + 1) * BL],
                                    lhsT=whhs[dir_i][:, g * 128 : (g + 1) * 128],
                                    rhs=hprevs[dir_i],
                                    start=False, stop=True,
                                )
                    # gate cols: [i f o 2g]; x2 on g is folded into the weights,
                    # so one sigmoid covers all four gates and
                    # tanh(g) = 2*sig(2g) - 1.
                    for dir_i in (0, 1):
                        sig = csb.tile([128, 64], F32, tag=f"sig{dir_i}", name="sig")
                        nc.scalar.activation(sig[:], gpss[dir_i][:], AF.Sigmoid)
                        sigs.append(sig)
                    cns = []
                    for dir_i in (0, 1):
                        sig = sigs[dir_i]
                        c_new = csb.tile([128, BL], F32, tag=f"c{dir_i}", name="c_new")
                        # k = si*(s2g - 1/2) = si*tanh(g)/2 ; c = 2k + u
                        if t > 0:
                            u = csb.tile([128, BL], F32, tag=f"u{dir_i}", name="u")
                            nc.vector.tensor_tensor(
                                out=u[:], in0=sig[:, BL : 2 * BL],
                                in1=c_slice[dir_i], op=ALU.mult,
                            )
                        k = csb.tile([128, BL], F32, tag=f"k{dir_i}", name="k")
                        nc.vector.scalar_tensor_tensor(
                            out=k[:], in0=sig[:, 3 * BL : 4 * BL], scalar=-0.5,
                            in1=sig[:, 0:BL], op0=ALU.add, op1=ALU.mult,
                        )
                        if t == 0:
                            nc.vector.tensor_scalar_mul(c_new[:], k[:], 2.0)
                        else:
                            nc.vector.scalar_tensor_tensor(
                                out=c_new[:], in0=k[:], scalar=2.0, in1=u[:],
                                op0=ALU.mult, op1=ALU.add,
                            )
                        cns.append(c_new)
                    tcs = []
                    for dir_i in (0, 1):
                        # sig(2c); h' = so*(sig(2c) - 1/2) = h/2, the x2 is
                        # folded into whh/wout host-side
                        tc_t = csb.tile([128, BL], F32, tag=f"tct{dir_i}", name="tc_t")
                        nc.scalar.activation(tc_t[:], cns[dir_i][:], AF.Sigmoid,
                                             scale=2.0)
                        tcs.append(tc_t)
                    for dir_i in (0, 1):
                        tt = tts[dir_i]
                        nc.vector.scalar_tensor_tensor(
                            out=hsts[dir_i][:, tt * BL : (tt + 1) * BL],
                            in0=tcs[dir_i][:], scalar=-0.5,
                            in1=sigs[dir_i][:, 2 * BL : 3 * BL],
                            op0=ALU.add, op1=ALU.mult,
                        )
                        c_slice[dir_i] = cns[dir_i][:]

                # ---------- tail: em chunks 0/15, start/end, CRF DP ----------
                for it in em_items(0):
                    it()
                for it in em_items(15):
                    it()
                tmp_s = esb.tile([T, BL], F32, tag="part", name="tmp_s")
                nc.vector.tensor_scalar_mul(tmp_s[:], oh[:, 0:BL], start_t[:])
                nc.gpsimd.tensor_tensor(
                    out=num_acc[:], in0=num_acc[:], in1=tmp_s[:], op=ALU.add
                )
                tmp_e = esb.tile([T, BL], F32, tag="part", name="tmp_e")
                nc.vector.tensor_scalar_mul(
                    tmp_e[:], oh[:, TB - BL : TB], end_t[:]
                )
                nc.gpsimd.tensor_tensor(
                    out=num_acc[:], in0=num_acc[:], in1=tmp_e[:], op=ALU.add
                )

                a0 = dsb.tile([T, BL], F32, tag="al", name="a0")
                nc.vector.tensor_scalar_mul(a0[:], expE[:, 0:BL], exp_start[:])
                g0 = dsb.tile([T, BL], F32, tag="ga", name="g0")
                nc.vector.tensor_scalar_mul(
                    g0[:], expE[:, (S - 1) * BL :][:, :BL], exp_end[:]
                )
                a_cur, g_cur = a0, g0
                na = ng = 0
                for j in range(S // 2 - 1):
                    sa, sg = 1 + j, (S - 2) - j
                    aps = dps.tile([T, BL], F32, tag="dpa", name="aps",
                                   space="PSUM")
                    nc.tensor.matmul(
                        aps[:], lhsT=expT[:], rhs=a_cur[:], start=True, stop=True
                    )
                    if sa % RENORM == 0:
                        e_sl = esc[:, (sa // RENORM) * BL :][:, :BL]
                        na += 1
                    else:
                        e_sl = expE[:, sa * BL : (sa + 1) * BL]
                    a_new = dsb.tile([T, BL], F32, tag="al", name="a_new")
                    nc.vector.tensor_tensor(
                        out=a_new[:], in0=aps[:], in1=e_sl, op=ALU.mult
                    )
                    a_cur = a_new
                    gp = dps.tile([T, BL], F32, tag="dpg", name="gp",
                                  space="PSUM")
                    nc.tensor.matmul(
                        gp[:], lhsT=expTT[:], rhs=g_cur[:], start=True, stop=True
                    )
                    if sg % RENORM == 0:
                        e_sl2 = esc[:, (sg // RENORM) * BL :][:, :BL]
                        ng += 1
                    else:
                        e_sl2 = expE[:, sg * BL : (sg + 1) * BL]
                    g_new = dsb.tile([T, BL], F32, tag="ga", name="g_new")
                    nc.vector.tensor_tensor(
                        out=g_new[:], in0=gp[:], in1=e_sl2, op=ALU.mult
                    )
                    g_cur = g_new

                # combine: denom = ln(sum_i gamma_256[i]*(M^T alpha_255)[i]) + C
                fps = dps.tile([T, BL], F32, tag="dpa", name="fps", space="PSUM")
                nc.tensor.matmul(
                    fps[:], lhsT=expT[:], rhs=a_cur[:], start=True, stop=True
                )
                fin = dsb.tile([T, BL], F32, tag="fin", name="fin")
                nc.vector.tensor_tensor(
                    out=fin[:], in0=fps[:], in1=g_cur[:], op=ALU.mult
                )
                sps = dps.tile([1, BL], F32, tag="dpa", name="sps", space="PSUM")
                nc.tensor.matmul(
                    sps[:], lhsT=rs_t1[:], rhs=fin[:], start=True, stop=True
                )
                lnf = dsb.tile([1, BL], F32, tag="lnf", name="lnf")
                nc.scalar.activation(lnf[:], sps[:], AF.Ln)
                scp = dps.tile([1, BL], F32, tag="dpg", name="scp", space="PSUM")
                nc.tensor.matmul(
                    scp[:], lhsT=ones_t1[:], rhs=num_acc[:], start=True, stop=True
                )
                C = (na + ng + 1) * RSH * math.log(2.0)
                res = dsb.tile([1, BL], F32, tag="res", name="res")
                nc.vector.scalar_tensor_tensor(
                    out=res[:], in0=scp[:], scalar=-C, in1=lnf[:],
                    op0=ALU.add, op1=ALU.subtract,
                )
                nc.sync.dma_start(out=out_d[:], in_=res[:])
                if DBG:
                    nc.sync.dma_start(out=dbg_fin[:], in_=fin[:])
                    nc.sync.dma_start(out=dbg_lnf[:], in_=lnf[:])
                    scpc = dsb.tile([1, BL], F32, tag="scpc", name="scpc")
                    nc.vector.tensor_copy(scpc[:], scp[:])
                    nc.sync.dma_start(out=dbg_scp[:], in_=scpc[:])
                    dtile = dsb.tile([128, 4 * BL], F32, tag="dbg", name="dtile")
                    nc.vector.tensor_copy(dtile[:], hf[:, 0 : 4 * BL])
                    nc.sync.dma_start(out=dbg_hf[:], in_=dtile[:])
                    dtile2 = dsb.tile([128, 4 * BL], F32, tag="dbg", name="dtile2")
                    nc.vector.tensor_copy(dtile2[:], hb[:, 0 : 4 * BL])
                    nc.sync.dma_start(out=dbg_hb[:], in_=dtile2[:])
                    nc.sync.dma_start(out=dbg_ee[:], in_=expE[:, 0 : 4 * BL])
                    nc.sync.dma_start(out=dbg_na[:], in_=num_acc[:])
                    nc.sync.dma_start(out=dbg_al[:], in_=a_cur[:])
                    nc.sync.dma_start(out=dbg_ga[:], in_=g_cur[:])
                    dtile3 = dsb.tile([T, 4 * BL], F32, tag="dbg2", name="dtile3")
                    nc.vector.tensor_copy(dtile3[:], oh[:, 0 : 4 * BL])
                    nc.sync.dma_start(out=dbg_oh[:], in_=dtile3[:])

    nc.compile()
    return nc


def make_in_maps(inputs, ncores=NCORES):
    """Shard full inputs into per-core in_maps (host-side layout prep only)."""
    import ml_dtypes

    BF = ml_dtypes.bfloat16
    x = np.asarray(inputs["x"], np.int32)
    tags = np.asarray(inputs["tags"], np.int32)
    emb = np.asarray(inputs["emb"], np.float32).astype(BF)
    xsT_all = np.empty((NCORES, E + 1, TB), BF)
    for c in range(NCORES):
        xe = emb[x[c * BL : (c + 1) * BL]]          # [BL, S, E] bf16
        xsT_all[c, :E] = xe.transpose(2, 1, 0).reshape(E, TB)
        xsT_all[c, E] = np.ones((TB,), BF)

    def reorder(w):
        # PyTorch gate order (i, f, g, o) -> kernel order (i, f, o, 2g); the
        # x2 on the g block makes one sigmoid serve all gates via
        # tanh(x) = 2*sigmoid(2x) - 1.
        wi, wf, wg, wo = np.split(np.asarray(w, np.float32), 4, axis=0)
        return np.concatenate([wi, wf, wo, 2.0 * wg], 0)

    def aug(w_ih, b):
        w = reorder(w_ih)          # [4H, E]
        bb = reorder(np.asarray(b, np.float32)[:, None])  # [4H, 1]
        return np.ascontiguousarray(
            np.concatenate([w.T, bb.T], 0).astype(BF)
        )  # [E+1, 4H]

    wih_f = aug(inputs["w_ih_f"], inputs["b_f"])
    wih_b = aug(inputs["w_ih_b"], inputs["b_b"])
    # h is stored as h/2 (h' = so*(sig(2c)-1/2)); fold the x2 into consumers
    whh_f = np.ascontiguousarray((2.0 * reorder(inputs["w_hh_f"])).T.astype(BF))
    whh_b = np.ascontiguousarray((2.0 * reorder(inputs["w_hh_b"])).T.astype(BF))
    W_out = 2.0 * np.asarray(inputs["W_out"], np.float32)
    wout = np.ascontiguousarray(
        np.concatenate([W_out[:, :H].T, W_out[:, H:].T], 1).astype(BF)
    )
    bout = np.ascontiguousarray(np.asarray(inputs["b_out"], np.float32)[:, None])
    start_c = np.ascontiguousarray(
        np.asarray(inputs["start_trans"], np.float32)[:, None]
    )
    end_c = np.ascontiguousarray(np.asarray(inputs["end_trans"], np.float32)[:, None])
    trans = np.ascontiguousarray(np.asarray(inputs["trans"], np.float32))
    transT = np.ascontiguousarray(trans.T)
    trans_bf = np.ascontiguousarray(trans.astype(BF))

    in_maps = []
    for c in range(ncores):
        tg = tags[c * BL : (c + 1) * BL]
        tags_tb = np.ascontiguousarray(
            tg.T.reshape(1, -1).astype(np.float32).astype(BF)
        )  # t-major [1, S*BL]
        in_maps.append(
            {
                "xsT_in": np.ascontiguousarray(xsT_all[c]),
                "tags_tb": tags_tb,
                "wihT_f": wih_f,
                "wihT_b": wih_b,
                "whhT_f": whh_f,
                "whhT_b": whh_b,
                "woutT": wout,
                "b_out_c": bout,
                "start_c": start_c,
                "end_c": end_c,
                "trans": trans,
                "transT": transT,
                "trans_bf": trans_bf,
            }
        )
    return in_maps


_NC_CACHE = {}


def _install_ntff_hook_shim():
    """The agent image's antenv lacks axon_hooks; replicate the ctypes NTFF
    profile hook (see trn_agent_boot/trn_boot.py) so trace=True works."""
    import contextlib
    import ctypes
    import types

    if "antenv.axon_hooks" in sys.modules:
        return
    so_path = "/opt/axon/libaxon_pjrt.so"
    try:
        lib = ctypes.CDLL(so_path)
    except OSError:
        return
    if not hasattr(lib, "axon_start_nrt_profile"):
        return
    lib.axon_start_nrt_profile.argtypes = [
        ctypes.POINTER(ctypes.c_int64),
        ctypes.c_size_t,
    ]
    lib.axon_start_nrt_profile.restype = ctypes.c_int64
    lib.axon_stop_nrt_profile.argtypes = [ctypes.c_char_p]
    lib.axon_stop_nrt_profile.restype = ctypes.c_int64

    @contextlib.contextmanager
    def _hook(output_dir, device_ids):
        import jax

        jax.devices()
        if device_ids:
            ids = (ctypes.c_int64 * len(device_ids))(*device_ids)
            rc = lib.axon_start_nrt_profile(ids, len(device_ids))
        else:
            rc = lib.axon_start_nrt_profile(None, 0)
        if rc != 0:
            raise RuntimeError(f"axon_start_nrt_profile rc={rc}")
        try:
            yield
        finally:
            n = lib.axon_stop_nrt_profile(str(output_dir).encode())
            print(f"profile: {n} file(s) written to {output_dir}")

    mod = types.ModuleType("antenv.axon_hooks")
    mod.get_axon_ntff_profile_hook = lambda: _hook
    mod.set_axon_ntff_profile_hook = lambda h: None
    sys.modules["antenv.axon_hooks"] = mod


def kernel(**inputs):
    from concourse.bass_utils import run_bass_kernel_spmd

    if "nc" not in _NC_CACHE:
        _NC_CACHE["nc"] = build_program()
    nc = _NC_CACHE["nc"]
    in_maps = make_in_maps(inputs)
    trace = bool(int(os.environ.get("BASS_KERNEL_TRACE", "0")))
    if trace:
        _install_ntff_hook_shim()
        import concourse.bass_utils as _bu

        _orig_upload = _bu.upload_artifacts

        def _safe_upload(tmpdir):
            try:
                return _orig_upload(tmpdir)
            except Exception as e:
                print(f"upload_artifacts failed ({e}); using local dir")
                return tmpdir

        _bu.upload_artifacts = _safe_upload
    res = run_bass_kernel_spmd(
        nc, in_maps, core_ids=list(range(NCORES)), trace=trace
    )
    if trace and res.exec_time_ns is not None:
        print(f"HW exec time: {res.exec_time_ns} ns")
    parts = np.concatenate([r["out"].reshape(-1) for r in res.results])
    return np.float32(-np.mean(parts))


# revision 37
# speedup vs baseline: 1.0020x; 1.0020x over previous
"""BiLSTM-CRF loss kernel for Trainium2, data-parallel over batch on 8 NeuronCores.

Per-core program (B_local=16 sequences, S=512, T=20 tags, E=100, H=128):
  Host prep: embeddings pre-gathered/transposed to xsT [E+1, S*16] bf16 (ones
  row folds the input-projection bias into the matmul); weight layouts
  transposed, gate order (i,f,o,g); the g block and whh/wout carry extra
  factors of 2 so the cell uses only Sigmoid activations and stores h/2.

  Main loop: 512-step fwd+bwd LSTM recurrence (two independent dependency
  chains) with producer work streamed in as background items between steps:
  xsT piece DMAs (on the Pool engine's DMA queue, off the Sync queue), the
  one-hot of tags, and (second half, middle-outward as hf/hb become
  available) the emission chunks em = W_out @ [hf;hb], expE = exp(em +
  b_out), and the CRF numerator pieces.

  Per step per dir: 4 small input-projection matmuls (K=E+1, N=16) write the
  next step's gates into a fresh PSUM bank one step ahead (they execute
  inside the h-wait stall on the PE); 4 W_hh matmuls then accumulate on top;
  one Sigmoid covers all gates (tanh(g) = 2*sig(2g)-1); cell update
  k = si*(s2g-1/2), c = 2k + sf*c_prev; tanh(c) via sig(2c);
  h' = h/2 = so*(sig(2c)-1/2). Measured cycle ~1.83us/step, at the
  dependency-chain floor for this cell structure.

  Tail: emission chunks 0/15, then the CRF partition function as two serial
  chains meeting in the middle: alpha (t=0..255) and gamma_t = E_t * beta_t
  (t=511..256), renormalized by the compile-time constant 2^-69 every 16
  steps (exact power of two, no data-dependent renorm work); the log2
  bookkeeping is added back as a constant in the final combine.

mask is all ones for this problem (spec fill=ones), so masking is elided and
seq_ends = S-1.
"""

import math
import os
import sys

import numpy as np

sys.path.insert(0, "/opt/trn_rl_repo")

import concourse.bass as bass  # noqa: F401 (registers bass types used by tile)
import concourse.mybir as mybir
import concourse.tile as tile
from concourse import bacc
from concourse.masks import make_identity

AF = mybir.ActivationFunctionType
ALU = mybir.AluOpType
AX = mybir.AxisListType
F32 = mybir.dt.float32
BF16 = mybir.dt.bfloat16
I32 = mybir.dt.int32

V, T, E, HD = 32000, 20, 100, 256
H = 128
B, S = 128, 512
NCORES = 8
BL = B // NCORES          # 16 sequences per core
TB = S * BL               # 8192 tokens per core
CHS = 32                  # time steps per projection/emission chunk
NPC = S // CHS            # 16 chunks
RENORM = 16               # DP renorm period (steps)
RSH = 69                  # A *= 2^-69 each renorm (~20^16)
DPH = S // 2              # alpha/gamma half length


def build_program():
    nc = bacc.Bacc(None, target_bir_lowering=False)

    # ---- DRAM I/O ----
    tags_d = nc.dram_tensor("tags_tb", [1, TB], BF16, kind="ExternalInput")
    CW = CHS * BL
    xsT_d = nc.dram_tensor("xsT_in", [E + 1, TB], BF16, kind="ExternalInput")
    wih_f_d = nc.dram_tensor("wihT_f", [E + 1, 4 * H], BF16, kind="ExternalInput")
    wih_b_d = nc.dram_tensor("wihT_b", [E + 1, 4 * H], BF16, kind="ExternalInput")
    whh_f_d = nc.dram_tensor("whhT_f", [H, 4 * H], BF16, kind="ExternalInput")
    whh_b_d = nc.dram_tensor("whhT_b", [H, 4 * H], BF16, kind="ExternalInput")
    wout_d = nc.dram_tensor("woutT", [H, 2 * T], BF16, kind="ExternalInput")
    bout_d = nc.dram_tensor("b_out_c", [T, 1], F32, kind="ExternalInput")
    start_d = nc.dram_tensor("start_c", [T, 1], F32, kind="ExternalInput")
    end_d = nc.dram_tensor("end_c", [T, 1], F32, kind="ExternalInput")
    trans_d = nc.dram_tensor("trans", [T, T], F32, kind="ExternalInput")
    transT_d = nc.dram_tensor("transT", [T, T], F32, kind="ExternalInput")
    trans_bf_d = nc.dram_tensor("trans_bf", [T, T], BF16, kind="ExternalInput")
    out_d = nc.dram_tensor("out", [1, BL], F32, kind="ExternalOutput")
    DBG = bool(int(os.environ.get("BASS_KERNEL_DEBUG", "0")))
    if DBG:
        dbg_hf = nc.dram_tensor("dbg_hf", [128, 4 * BL], F32, kind="ExternalOutput")
        dbg_hb = nc.dram_tensor("dbg_hb", [128, 4 * BL], F32, kind="ExternalOutput")
        dbg_ee = nc.dram_tensor("dbg_ee", [T, 4 * BL], F32, kind="ExternalOutput")
        dbg_na = nc.dram_tensor("dbg_na", [T, BL], F32, kind="ExternalOutput")
        dbg_al = nc.dram_tensor("dbg_al", [T, BL], F32, kind="ExternalOutput")
        dbg_ga = nc.dram_tensor("dbg_ga", [T, BL], F32, kind="ExternalOutput")
        dbg_oh = nc.dram_tensor("dbg_oh", [T, 4 * BL], F32, kind="ExternalOutput")
        dbg_fin = nc.dram_tensor("dbg_fin", [T, BL], F32, kind="ExternalOutput")
        dbg_lnf = nc.dram_tensor("dbg_lnf", [1, BL], F32, kind="ExternalOutput")
        dbg_scp = nc.dram_tensor("dbg_scp", [1, BL], F32, kind="ExternalOutput")

    with tile.TileContext(nc) as tc:
        with tc.tile_pool(name="persist", bufs=1) as pp:
            # ---- persistent SBUF tiles ----
            xps = []
            for pi in range(16):
                xp = pp.tile([E + 1, TB // 16], BF16, tag=f"xs{pi}",
                             name=f"xs{pi}")
                xps.append(xp)
            hf = pp.tile([128, TB], BF16, tag="hf")
            hb = pp.tile([128, TB], BF16, tag="hb")
            expE = pp.tile([T, TB], F32, tag="expE")
            esc = pp.tile([T, 2 * CHS * BL], F32, tag="esc")  # 2^-69-scaled slices
            oh = pp.tile([T, TB], BF16, tag="oh")
            wih_f = pp.tile([E + 1, 4 * H], BF16, tag="wihf")
            wih_b = pp.tile([E + 1, 4 * H], BF16, tag="wihb")
            whh_f = pp.tile([H, 4 * H], BF16, tag="whhf")
            whh_b = pp.tile([H, 4 * H], BF16, tag="whhb")
            wout = pp.tile([H, 2 * T], BF16, tag="wout")
            bout = pp.tile([T, 1], F32, tag="bout")
            start_t = pp.tile([T, 1], F32, tag="start")
            end_t = pp.tile([T, 1], F32, tag="end")
            trans_bf = pp.tile([T, T], BF16, tag="transbf")
            expT = pp.tile([T, T], F32, tag="expT")
            expTT = pp.tile([T, T], F32, tag="expTT")
            exp_end = pp.tile([T, 1], F32, tag="expend")
            exp_start = pp.tile([T, 1], F32, tag="expstart")
            identb = pp.tile([128, 128], BF16, tag="identb")
            ones_t1 = pp.tile([T, 1], F32, tag="onest1")
            rs_t1 = pp.tile([T, 1], F32, tag="rst1")  # 2^-69 column for the
            # final colsum so Ln's input lands in a sane range
            ones_1t = pp.tile([1, T], BF16, tag="ones1t")
            iot_f = pp.tile([T, 1], F32, tag="iotf")
            num_acc = pp.tile([T, BL], F32, tag="numacc")

            # ---- param loads & constants (spread across engine DMA queues
            # so the first recurrence steps aren't gated on one serial queue) ----
            tmp_tr = pp.tile([T, T], F32, tag="tmptr")
            for eng, sb, d in [
                (nc.sync, wih_f, wih_f_d), (nc.scalar, wih_b, wih_b_d),
                (nc.gpsimd, whh_f, whh_f_d), (nc.scalar, whh_b, whh_b_d),
                (nc.sync, wout, wout_d), (nc.sync, bout, bout_d),
                (nc.sync, start_t, start_d), (nc.sync, end_t, end_d),
                (nc.gpsimd, trans_bf, trans_bf_d),
            ]:
                eng.dma_start(out=sb[:], in_=d[:])
            nc.scalar.dma_start(out=tmp_tr[:], in_=trans_d[:])
            nc.scalar.activation(expT[:], tmp_tr[:], AF.Exp)
            tmp_tr2 = pp.tile([T, T], F32, tag="tmptr2")
            nc.scalar.dma_start(out=tmp_tr2[:], in_=transT_d[:])
            nc.scalar.activation(expTT[:], tmp_tr2[:], AF.Exp)
            nc.scalar.activation(exp_end[:], end_t[:], AF.Exp)
            nc.scalar.activation(exp_start[:], start_t[:], AF.Exp)
            make_identity(nc, identb[:])
            nc.vector.memset(ones_t1[:], 1.0)
            nc.vector.memset(rs_t1[:], float(2.0 ** (-RSH)))
            nc.vector.memset(ones_1t[:], 1.0)
            iot_i = pp.tile([T, 1], I32, tag="ioti")
            nc.gpsimd.iota(iot_i[:], pattern=[[0, 1]], base=0, channel_multiplier=1)
            nc.vector.tensor_copy(iot_f[:], iot_i[:])
            nc.vector.memset(num_acc[:], 0.0)

            with (
                tc.tile_pool(name="gat_sb", bufs=3) as gsb,
                tc.tile_pool(name="wide_ps", bufs=1, space="PSUM") as wps,
                tc.tile_pool(name="g_ps", bufs=3, space="PSUM") as gps_pool,
                tc.tile_pool(name="p_sb", bufs=3) as psb,
                tc.tile_pool(name="cell_sb", bufs=4) as csb,
                tc.tile_pool(name="em_sb", bufs=3) as esb,
                tc.tile_pool(name="dp_ps", bufs=2, space="PSUM") as dps,
                tc.tile_pool(name="dp_sb", bufs=3) as dsb,
            ):
                # ---------- background item emitters ----------

                def emit_xchunk(c):
                    # one xsT piece = one whole tile = one DMA writer, so each
                    # projection matmul waits on exactly one semaphore; issued
                    # from the idle Pool engine's DMA queue
                    cs = slice(c * TB // 16, (c + 1) * TB // 16)
                    nc.gpsimd.dma_start(out=xps[c][:], in_=xsT_d[:, cs])

                def emit_oh(c):
                    # one-hot of tags for chunk c (tags only; no recurrence dep)
                    cs = slice(c * CHS * BL, (c + 1) * CHS * BL)
                    tgc = gsb.tile([1, CW], BF16, tag="tgc", name="tgc")
                    nc.gpsimd.dma_start(out=tgc[:], in_=tags_d[:, cs])
                    wtile = wps.tile([128, 1024], BF16, tag="wide", name="wtile")
                    ohp = wtile[0:T, :].bitcast(F32)
                    nc.tensor.matmul(
                        ohp, lhsT=ones_1t[:], rhs=tgc[:],
                        start=True, stop=True,
                    )
                    nc.vector.tensor_tensor(
                        out=oh[:, cs], in0=ohp,
                        in1=iot_f[:].to_broadcast([T, CHS * BL]), op=ALU.is_equal,
                    )

                RS = float(2.0 ** (-RSH))
                # ---------- emission-chunk items ----------
                na_ng = [0, 0]

                def em_items(c):
                    CWc = CHS * BL
                    cs = slice(c * CWc, (c + 1) * CWc)
                    st = {}

                    def i_mm():
                        wtile = wps.tile([128, 1024], BF16, tag="wide",
                                         name="wtile")
                        emp = wtile[0:T, :].bitcast(F32)
                        st["emp"] = emp
                        h2 = CWc // 2
                        cs_a = slice(c * CWc, c * CWc + h2)
                        nc.tensor.matmul(
                            emp[:, :h2], lhsT=wout[:, 0:T], rhs=hf[:, cs_a],
                            start=True, stop=False,
                        )
                        nc.tensor.matmul(
                            emp[:, :h2], lhsT=wout[:, T : 2 * T], rhs=hb[:, cs_a],
                            start=False, stop=True,
                        )

                    def i_mm_b():
                        h2 = CWc // 2
                        cs_b = slice(c * CWc + h2, (c + 1) * CWc)
                        emp = st["emp"]
                        nc.tensor.matmul(
                            emp[:, h2:], lhsT=wout[:, 0:T], rhs=hf[:, cs_b],
                            start=True, stop=False,
                        )
                        nc.tensor.matmul(
                            emp[:, h2:], lhsT=wout[:, T : 2 * T], rhs=hb[:, cs_b],
                            start=False, stop=True,
                        )

                    def i_exp():
                        # expE = exp(em + b_out) (bias folded into activation)
                        nc.scalar.activation(expE[:, cs], st["emp"], AF.Exp,
                                             bias=bout[:])

                    def i_esc():
                        for sI in range(c * CHS, (c + 1) * CHS):
                            if sI % RENORM == 0 and sI >= RENORM:
                                col = (sI // RENORM) * BL
                                nc.gpsimd.tensor_scalar_mul(
                                    esc[:, col : col + BL],
                                    expE[:, sI * BL : (sI + 1) * BL], RS,
                                )

                    def i_prod_a():
                        prod = esb.tile([T, CWc], F32, tag="prod", name="prod")
                        st["prod"] = prod
                        nc.vector.scalar_tensor_tensor(
                            out=prod[:, : CWc // 2], in0=st["emp"][:, : CWc // 2],
                            scalar=bout[:],
                            in1=oh[:, c * CWc : c * CWc + CWc // 2],
                            op0=ALU.add, op1=ALU.mult,
                        )

                    def i_prod_b():
                        nc.vector.scalar_tensor_tensor(
                            out=st["prod"][:, CWc // 2 :],
                            in0=st["emp"][:, CWc // 2 :], scalar=bout[:],
                            in1=oh[:, c * CWc + CWc // 2 : (c + 1) * CWc],
                            op0=ALU.add, op1=ALU.mult,
                        )

                    def i_red_a():
                        part = esb.tile([T, BL], F32, tag="part", name="part")
                        st["part"] = part
                        nc.vector.reduce_sum(
                            part[:],
                            st["prod"][:, : CWc // 2].rearrange(
                                "p (t b) -> p b t", b=BL
                            ),
                            axis=AX.X,
                        )

                    def i_red_b():
                        part_b = esb.tile([T, BL], F32, tag="part", name="part_b")
                        nc.vector.reduce_sum(
                            part_b[:],
                            st["prod"][:, CWc // 2 :].rearrange(
                                "p (t b) -> p b t", b=BL
                            ),
                            axis=AX.X,
                        )
                        nc.gpsimd.tensor_tensor(
                            out=num_acc[:], in0=num_acc[:], in1=st["part"][:],
                            op=ALU.add,
                        )
                        nc.gpsimd.tensor_tensor(
                            out=num_acc[:], in0=num_acc[:], in1=part_b[:],
                            op=ALU.add,
                        )

                    def i_trp():
                        wtile = wps.tile([128, 1024], BF16, tag="wide",
                                         name="wtile")
                        trp = wtile[0:T, :].bitcast(F32)
                        st["trp"] = trp
                        h2 = CWc // 2
                        nc.tensor.matmul(
                            trp[:, :h2], lhsT=trans_bf[:],
                            rhs=oh[:, c * CWc : c * CWc + h2],
                            start=True, stop=True,
                        )
                        nc.tensor.matmul(
                            trp[:, h2:], lhsT=trans_bf[:],
                            rhs=oh[:, c * CWc + h2 : (c + 1) * CWc],
                            start=True, stop=True,
                        )

                    def i_prod2_a():
                        npair = CHS if c < NPC - 1 else CHS - 1
                        st["npair"] = npair
                        prod2 = esb.tile([T, CWc], F32, tag="prod", name="prod2")
                        st["prod2"] = prod2
                        nc.vector.tensor_tensor(
                            out=prod2[:, : (npair * BL) // 2],
                            in0=st["trp"][:, : (npair * BL) // 2],
                            in1=oh[:, c * CWc + BL :][:, : (npair * BL) // 2],
                            op=ALU.mult,
                        )

                    def i_prod2_b():
                        npair = st["npair"]
                        h0 = (npair * BL) // 2
                        nc.vector.tensor_tensor(
                            out=st["prod2"][:, h0 : npair * BL],
                            in0=st["trp"][:, h0 : npair * BL],
                            in1=oh[:, c * CWc + BL + h0 :][
                                :, : npair * BL - h0
                            ],
                            op=ALU.mult,
                        )

                    def i_red2():
                        part2 = esb.tile([T, BL], F32, tag="part", name="part2")
                        nc.vector.reduce_sum(
                            part2[:],
                            st["prod2"][:, : st["npair"] * BL].rearrange(
                                "p (t b) -> p b t", b=BL
                            ),
                            axis=AX.X,
                        )
                        nc.gpsimd.tensor_tensor(
                            out=num_acc[:], in0=num_acc[:], in1=part2[:],
                            op=ALU.add,
                        )

                    return [i_mm, i_mm_b, i_exp, i_esc, i_prod_a, i_prod_b, i_red_a,
                            i_red_b, i_trp, i_prod2_a, i_prod2_b, i_red2]

                # ---------- background schedule ----------
                prologue = [("xch", 0), ("xch", 15)]
                windows = {i: [] for i in range(1, 16)}
                for i in range(1, 15):
                    # xsT piece i feeds proj chunk i (consumed from window i+1)
                    windows[i].append(("xch", i))
                    windows[i].append(("xch", 15 - i))
                for c in range(NPC):
                    windows[(c % 15) + 1].append(("oh", c))
                windows[16] = []
                # em chunks become computable middle-outward as hf/hb meet;
                # chunks 0 and 15 only at the very end (tail handles those)
                for wi, (ca, cb) in zip(
                    range(10, 17),
                    [(7, 8), (6, 9), (5, 10), (4, 11), (3, 12), (2, 13), (1, 14)],
                ):
                    for it in em_items(ca):
                        windows[wi].append(("emi", it))
                    for it in em_items(cb):
                        windows[wi].append(("emi", it))

                def run_item(item):
                    if item[0] == "xch":
                        emit_xchunk(item[1])
                    elif item[0] == "emi":
                        item[1]()
                    else:
                        emit_oh(item[1])

                wihs = [wih_f, wih_b]

                def emit_step_proj(dir_i, tt, final):
                    # input projection for step tt straight into a fresh gates
                    # PSUM bank; the whh matmuls later accumulate on top
                    gp = gps_pool.tile([128, 64], F32, tag="g", name="g_ps",
                                       space="PSUM")
                    xp = xps[tt // CHS]
                    xo = (tt % CHS) * BL
                    for g in range(4):
                        nc.tensor.matmul(
                            gp[:, g * BL : (g + 1) * BL],
                            lhsT=wihs[dir_i][:, g * 128 : (g + 1) * 128],
                            rhs=xp[:, xo : xo + BL],
                            start=True, stop=final,
                        )
                    return gp

                next_gps = [None, None]

                # PE p-state warm-up: ~5us of continuous dependency-free
                # matmuls on the on-device identity (no DMA input needed)
                # while the weight/xsT DMAs are still in flight, so the
                # Tensor engine reaches full clock before step 0
                warm = wps.tile([128, 1024], BF16, tag="wide", name="warm")
                wv = warm[:].bitcast(F32)[:, 0:128]
                for _ in range(24):
                    nc.tensor.matmul(wv, lhsT=identb[:], rhs=identb[:],
                                     start=True, stop=True)
                for item in prologue:
                    run_item(item)
                next_gps[0] = emit_step_proj(0, 0, True)
                next_gps[1] = emit_step_proj(1, S - 1, True)

                # ---------- main recurrence ----------
                c_slice = {0: None, 1: None}
                wq, wlen, qi = [], 0, 0
                for t in range(S):
                    if t % CHS == 0:
                        wq = windows.get(t // CHS + 1, [])
                        wlen, qi = len(wq), 0
                    # spread this window's items over its 32 steps
                    target = ((t % CHS) + 1) * wlen // CHS
                    while qi < target:
                        run_item(wq[qi])
                        qi += 1
                    tb_ = S - 1 - t
                    tts, whhs, hsts, hprevs, gpss, sigs = [], [], [], [], [], []
                    for dir_i in (0, 1):
                        if dir_i == 0:
                            tts.append(t)
                            whhs.append(whh_f)
                            hsts.append(hf)
                            hprevs.append(
                                None if t == 0 else hf[:, (t - 1) * BL : t * BL]
                            )
                        else:
                            tts.append(tb_)
                            whhs.append(whh_b)
                            hsts.append(hb)
                            hprevs.append(
                                None if t == 0
                                else hb[:, (tb_ + 1) * BL : (tb_ + 2) * BL]
                            )
                    for dir_i in (0, 1):
                        g_ps = next_gps[dir_i]
                        gpss.append(g_ps)
                        if t + 1 < S:
                            tt2 = t + 1 if dir_i == 0 else S - 2 - t
                            next_gps[dir_i] = emit_step_proj(dir_i, tt2, False)
                        if t > 0:
                            for g in range(4):
                                nc.tensor.matmul(
                                    g_ps[:, g * BL : (g + 1) * BL],
                                    lhsT=whhs[dir_i][:, g * 128 : (g + 1) * 128],
                                    rhs=hprevs[dir_i],
                                    start=False, stop=True,
                                )
                    # gate cols: [i f o 2g]; x2 on g is folded into the weights,
                    # so one sigmoid covers all four gates and
                    # tanh(g) = 2*sig(2g) - 1.
                    for dir_i in (0, 1):
                        sig = csb.tile([128, 64], F32, tag=f"sig{dir_i}", name="sig")
                        nc.scalar.activation(sig[:], gpss[dir_i][:], AF.Sigmoid)
                        sigs.append(sig)
                    cns = []
                    for dir_i in (0, 1):
                        sig = sigs[dir_i]
                        c_new = csb.tile([128, BL], F32, tag=f"c{dir_i}", name="c_new")
                        # k = si*(s2g - 1/2) = si*tanh(g)/2 ; c = 2k + u
                        if t > 0:
                            u = csb.tile([128, BL], F32, tag=f"u{dir_i}", name="u")
                            nc.vector.tensor_tensor(
                                out=u[:], in0=sig[:, BL : 2 * BL],
                                in1=c_slice[dir_i], op=ALU.mult,
                            )
                        k = csb.tile([128, BL], F32, tag=f"k{dir_i}", name="k")
                        nc.vector.scalar_tensor_tensor(
                            out=k[:], in0=sig[:, 3 * BL : 4 * BL], scalar=-0.5,
                            in1=sig[:, 0:BL], op0=ALU.add, op1=ALU.mult,
                        )
                        if t == 0:
                            nc.vector.tensor_scalar_mul(c_new[:], k[:], 2.0)
                        else:
                            nc.vector.scalar_tensor_tensor(
                                out=c_new[:], in0=k[:], scalar=2.0, in1=u[:],
                                op0=ALU.mult, op1=ALU.add,
                            )
                        cns.append(c_new)
                    tcs = []
                    for dir_i in (0, 1):
                        # sig(2c); h' = so*(sig(2c) - 1/2) = h/2, the x2 is
                        # folded into whh/wout host-side
                        tc_t = csb.tile([128, BL], F32, tag=f"tct{dir_i}", name="tc_t")
                        nc.scalar.activation(tc_t[:], cns[dir_i][:], AF.Sigmoid,
                                             scale=2.0)
                        tcs.append(tc_t)
                    for dir_i in (0, 1):
                        tt = tts[dir_i]
                        nc.vector.scalar_tensor_tensor(
                            out=hsts[dir_i][:, tt * BL : (tt + 1) * BL],
                            in0=tcs[dir_i][:], scalar=-0.5,
                            in1=sigs[dir_i][:, 2 * BL : 3 * BL],
                            op0=ALU.add, op1=ALU.mult,
                        )
                        c_slice[dir_i] = cns[dir_i][:]

                # ---------- tail: em chunks 0/15, start/end, CRF DP ----------
                for it in em_items(0):
                    it()
                for it in em_items(15):
                    it()
                tmp_s = esb.tile([T, BL], F32, tag="part", name="tmp_s")
                nc.vector.tensor_scalar_mul(tmp_s[:], oh[:, 0:BL], start_t[:])
                nc.gpsimd.tensor_tensor(
                    out=num_acc[:], in0=num_acc[:], in1=tmp_s[:], op=ALU.add
                )
                tmp_e = esb.tile([T, BL], F32, tag="part", name="tmp_e")
                nc.vector.tensor_scalar_mul(
                    tmp_e[:], oh[:, TB - BL : TB], end_t[:]
                )
                nc.gpsimd.tensor_tensor(
                    out=num_acc[:], in0=num_acc[:], in1=tmp_e[:], op=ALU.add
                )

                a0 = dsb.tile([T, BL], F32, tag="al", name="a0")
                nc.vector.tensor_scalar_mul(a0[:], expE[:, 0:BL], exp_start[:])
                g0 = dsb.tile([T, BL], F32, tag="ga", name="g0")
                nc.vector.tensor_scalar_mul(
                    g0[:], expE[:, (S - 1) * BL :][:, :BL], exp_end[:]
                )
                a_cur, g_cur = a0, g0
                na = ng = 0
                for j in range(S // 2 - 1):
                    sa, sg = 1 + j, (S - 2) - j
                    aps = dps.tile([T, BL], F32, tag="dpa", name="aps",
                                   space="PSUM")
                    nc.tensor.matmul(
                        aps[:], lhsT=expT[:], rhs=a_cur[:], start=True, stop=True
                    )
                    if sa % RENORM == 0:
                        e_sl = esc[:, (sa // RENORM) * BL :][:, :BL]
                        na += 1
                    else:
                        e_sl = expE[:, sa * BL : (sa + 1) * BL]
                    a_new = dsb.tile([T, BL], F32, tag="al", name="a_new")
                    nc.vector.tensor_tensor(
                        out=a_new[:], in0=aps[:], in1=e_sl, op=ALU.mult
                    )
                    a_cur = a_new
                    gp = dps.tile([T, BL], F32, tag="dpg", name="gp",
                                  space="PSUM")
                    nc.tensor.matmul(
                        gp[:], lhsT=expTT[:], rhs=g_cur[:], start=True, stop=True
                    )
                    if sg % RENORM == 0:
                        e_sl2 = esc[:, (sg // RENORM) * BL :][:, :BL]
                        ng += 1
                    else:
                        e_sl2 = expE[:, sg * BL : (sg + 1) * BL]
                    g_new = dsb.tile([T, BL], F32, tag="ga", name="g_new")
                    nc.vector.tensor_tensor(
                        out=g_new[:], in0=gp[:], in1=e_sl2, op=ALU.mult
                    )
                    g_cur = g_new

                # combine: denom = ln(sum_i gamma_256[i]*(M^T alpha_255)[i]) + C
                fps = dps.tile([T, BL], F32, tag="dpa", name="fps", space="PSUM")
                nc.tensor.matmul(
                    fps[:], lhsT=expT[:], rhs=a_cur[:], start=True, stop=True
                )
                fin = dsb.tile([T, BL], F32, tag="fin", name="fin")
                nc.vector.tensor_tensor(
                    out=fin[:], in0=fps[:], in1=g_cur[:], op=ALU.mult
                )
                sps = dps.tile([1, BL], F32, tag="dpa", name="sps", space="PSUM")
                nc.tensor.matmul(
                    sps[:], lhsT=rs_t1[:], rhs=fin[:], start=True, stop=True
                )
                lnf = dsb.tile([1, BL], F32, tag="lnf", name="lnf")
                nc.scalar.activation(lnf[:], sps[:], AF.Ln)
                scp = dps.tile([1, BL], F32, tag="dpg", name="scp", space="PSUM")
                nc.tensor.matmul(
                    scp[:], lhsT=ones_t1[:], rhs=num_acc[:], start=True, stop=True
                )
                C = (na + ng + 1) * RSH * math.log(2.0)
                res = dsb.tile([1, BL], F32, tag="res", name="res")
                nc.vector.scalar_tensor_tensor(
                    out=res[:], in0=scp[:], scalar=-C, in1=lnf[:],
                    op0=ALU.add, op1=ALU.subtract,
                )
                nc.sync.dma_start(out=out_d[:], in_=res[:])
                if DBG:
                    nc.sync.dma_start(out=dbg_fin[:], in_=fin[:])
                    nc.sync.dma_start(out=dbg_lnf[:], in_=lnf[:])
                    scpc = dsb.tile([1, BL], F32, tag="scpc", name="scpc")
                    nc.vector.tensor_copy(scpc[:], scp[:])
                    nc.sync.dma_start(out=dbg_scp[:], in_=scpc[:])
                    dtile = dsb.tile([128, 4 * BL], F32, tag="dbg", name="dtile")
                    nc.vector.tensor_copy(dtile[:], hf[:, 0 : 4 * BL])
                    nc.sync.dma_start(out=dbg_hf[:], in_=dtile[:])
                    dtile2 = dsb.tile([128, 4 * BL], F32, tag="dbg", name="dtile2")
                    nc.vector.tensor_copy(dtile2[:], hb[:, 0 : 4 * BL])
                    nc.sync.dma_start(out=dbg_hb[:], in_=dtile2[:])
                    nc.sync.dma_start(out=dbg_ee[:], in_=expE[:, 0 : 4 * BL])
                    nc.sync.dma_start(out=dbg_na[:], in_=num_acc[:])
                    nc.sync.dma_start(out=dbg_al[:], in_=a_cur[:])
                    nc.sync.dma_start(out=dbg_ga[:], in_=g_cur[:])
                    dtile3 = dsb.tile([T, 4 * BL], F32, tag="dbg2", name="dtile3")
                    nc.vector.tensor_copy(dtile3[:], oh[:, 0 : 4 * BL])
                    nc.sync.dma_start(out=dbg_oh[:], in_=dtile3[:])

    nc.compile()
    return nc


def make_in_maps(inputs, ncores=NCORES):
    """Shard full inputs into per-core in_maps (host-side layout prep only)."""
    import ml_dtypes

    BF = ml_dtypes.bfloat16
    x = np.asarray(inputs["x"], np.int32)
    tags = np.asarray(inputs["tags"], np.int32)
    emb = np.asarray(inputs["emb"], np.float32).astype(BF)
    xsT_all = np.empty((NCORES, E + 1, TB), BF)
    for c in range(NCORES):
        xe = emb[x[c * BL : (c + 1) * BL]]          # [BL, S, E] bf16
        xsT_all[c, :E] = xe.transpose(2, 1, 0).reshape(E, TB)
        xsT_all[c, E] = np.ones((TB,), BF)

    def reorder(w):
        # PyTorch gate order (i, f, g, o) -> kernel order (i, f, o, 2g); the
        # x2 on the g block makes one sigmoid serve all gates via
        # tanh(x) = 2*sigmoid(2x) - 1.
        wi, wf, wg, wo = np.split(np.asarray(w, np.float32), 4, axis=0)
        return np.concatenate([wi, wf, wo, 2.0 * wg], 0)

    def aug(w_ih, b):
        w = reorder(w_ih)          # [4H, E]
        bb = reorder(np.asarray(b, np.float32)[:, None])  # [4H, 1]
        return np.ascontiguousarray(
            np.concatenate([w.T, bb.T], 0).astype(BF)
        )  # [E+1, 4H]

    wih_f = aug(inputs["w_ih_f"], inputs["b_f"])
    wih_b = aug(inputs["w_ih_b"], inputs["b_b"])
    # h is stored as h/2 (h' = so*(sig(2c)-1/2)); fold the x2 into consumers
    whh_f = np.ascontiguousarray((2.0 * reorder(inputs["w_hh_f"])).T.astype(BF))
    whh_b = np.ascontiguousarray((2.0 * reorder(inputs["w_hh_b"])).T.astype(BF))
    W_out = 2.0 * np.asarray(inputs["W_out"], np.float32)
    wout = np.ascontiguousarray(
        np.concatenate([W_out[:, :H].T, W_out[:, H:].T], 1).astype(BF)
    )
    bout = np.ascontiguousarray(np.asarray(inputs["b_out"], np.float32)[:, None])
    start_c = np.ascontiguousarray(
        np.asarray(inputs["start_trans"], np.float32)[:, None]
    )
    end_c = np.ascontiguousarray(np.asarray(inputs["end_trans"], np.float32)[:, None])
    trans = np.ascontiguousarray(np.asarray(inputs["trans"], np.float32))
    transT = np.ascontiguousarray(trans.T)
    trans_bf = np.ascontiguousarray(trans.astype(BF))

    in_maps = []
    for c in range(ncores):
        tg = tags[c * BL : (c + 1) * BL]
        tags_tb = np.ascontiguousarray(
            tg.T.reshape(1, -1).astype(np.float32).astype(BF)
        )  # t-major [1, S*BL]
        in_maps.append(
            {
                "xsT_in": np.ascontiguousarray(xsT_all[c]),
                "tags_tb": tags_tb,
                "wihT_f": wih_f,
                "wihT_b": wih_b,
                "whhT_f": whh_f,
                "whhT_b": whh_b,
                "woutT": wout,
                "b_out_c": bout,
                "start_c": start_c,
                "end_c": end_c,
                "trans": trans,
                "transT": transT,
                "trans_bf": trans_bf,
            }
        )
    return in_maps


_NC_CACHE = {}


def _install_ntff_hook_shim():
    """The agent image's antenv lacks axon_hooks; replicate the ctypes NTFF
    profile hook (see trn_agent_boot/trn_boot.py) so trace=True works."""
    import contextlib
    import ctypes
    import types

    if "antenv.axon_hooks" in sys.modules:
        return
    so_path = "/opt/axon/libaxon_pjrt.so"
    try:
        lib = ctypes.CDLL(so_path)
    except OSError:
        return
    if not hasattr(lib, "axon_start_nrt_profile"):
        return
    lib.axon_start_nrt_profile.argtypes = [
        ctypes.POINTER(ctypes.c_int64),
        ctypes.c_size_t,
    ]
    lib.axon_start_nrt_profile.restype = ctypes.c_int64
    lib.axon_stop_nrt_profile.argtypes = [ctypes.c_char_p]
    lib.axon_stop_nrt_profile.restype = ctypes.c_int64

    @contextlib.contextmanager
    def _hook(output_dir, device_ids):
        import jax

        jax.devices()
        if device_ids:
            ids = (ctypes.c_int64 * len(device_ids))(*device_ids)
            rc = lib.axon_start_nrt_profile(ids, len(device_ids))
        else:
            rc = lib.axon_start_nrt_profile(None, 0)
        if rc != 0:
            raise RuntimeError(f"axon_start_nrt_profile rc={rc}")
        try:
            yield
        finally:
            n = lib.axon_stop_nrt_profile(str(output_dir).encode())
            print(f"profile: {n} file(s) written to {output_dir}")

    mod = types.ModuleType("antenv.axon_hooks")
    mod.get_axon_ntff_profile_hook = lambda: _hook
    mod.set_axon_ntff_profile_hook = lambda h: None
    sys.modules["antenv.axon_hooks"] = mod


def kernel(**inputs):
    from concourse.bass_utils import run_bass_kernel_spmd

    if "nc" not in _NC_CACHE:
        _NC_CACHE["nc"] = build_program()
    nc = _NC_CACHE["nc"]
    in_maps = make_in_maps(inputs)
    trace = bool(int(os.environ.get("BASS_KERNEL_TRACE", "0")))
    if trace:
        _install_ntff_hook_shim()
        import concourse.bass_utils as _bu

        _orig_upload = _bu.upload_artifacts

        def _safe_upload(tmpdir):
            try:
                return _orig_upload(tmpdir)
            except Exception as e:
                print(f"upload_artifacts failed ({e}); using local dir")
                return tmpdir

        _bu.upload_artifacts = _safe_upload
    res = run_bass_kernel_spmd(
        nc, in_maps, core_ids=list(range(NCORES)), trace=trace
    )
    if trace and res.exec_time_ns is not None:
        print(f"HW exec time: {res.exec_time_ns} ns")
    parts = np.concatenate([r["out"].reshape(-1) for r in res.results])
    return np.float32(-np.mean(parts))
